# revision 8
# baseline (speedup 1.0000x reference)
"""ContrastiveCenterLoss on 8 Trainium2 NeuronCores.

Math: with dist[b,c] = ||f_b - c_c||^2,
  intra = sum_b dist[b, label_b]          = sum_b ||f_b - c_{label_b}||^2
  total = sum_{b,c} dist[b,c]             = C*sum||f||^2 + B*sum||c||^2 - 2*(sum_b f_b)@(sum_c c_c)
  inter = total - intra
  loss  = (1/2/B) * intra / (inter + 1e-6) / 0.1

Sharding: feat/label batch-sharded (2048 rows/core); centers statistics
sharded over 512-row slices; the full centers table stays in HBM and is
row-gathered by label via indirect DMA. Host all-reduces the per-core
partial sums in float64 and applies the final scalar division.
"""

import numpy as np

B, C, D = 16384, 4096, 128
LAMBDA_C = 1.0
NCORES = 8
BS = B // NCORES          # 2048 feat rows per core
NPT = BS // 128           # 16 feat rows per partition
NCHUNK = 4                # feat processed in 4 chunks of 512 free-dim cols
CPC = NPT // NCHUNK       # 4 row-blocks per chunk
CS = C // NCORES          # 512 center rows per core (stats slice)
CSPT = CS // 128          # 4 center rows per partition

_cached = {}


def _build_nc(repeat=1, gather_mode="indirect"):
    import concourse.bass as bass
    import concourse.tile as tile
    from concourse import bacc, mybir

    f32 = mybir.dt.float32
    i32 = mybir.dt.int32

    nc = bacc.Bacc("TRN2", target_bir_lowering=False, debug=False,
                   num_devices=NCORES)

    feat = nc.dram_tensor("feat", [BS, D], f32, kind="ExternalInput")
    labt = nc.dram_tensor("labt", [128, NPT], i32, kind="ExternalInput")
    centers = nc.dram_tensor("centers", [C, D], f32, kind="ExternalInput")
    cslice = nc.dram_tensor("cslice", [CS, D], f32, kind="ExternalInput")

    o_fsq = nc.dram_tensor("o_fsq", [128, NCHUNK], f32, kind="ExternalOutput")
    o_intra = nc.dram_tensor("o_intra", [128, NCHUNK], f32, kind="ExternalOutput")
    o_csq = nc.dram_tensor("o_csq", [128, 1], f32, kind="ExternalOutput")
    o_vec = nc.dram_tensor("o_vec", [1, 1024], f32, kind="ExternalOutput")

    CW = CPC * D  # 512 free-dim columns per chunk

    with tile.TileContext(nc) as tc:
        with tc.tile_pool(name="const", bufs=1) as cpool, \
             tc.tile_pool(name="sbuf", bufs=2) as pool, \
             tc.tile_pool(name="scratch", bufs=2) as spool, \
             tc.tile_pool(name="psum", bufs=2, space="PSUM") as psum:

            ones = cpool.tile([128, 1], f32)
            nc.vector.memset(ones[:], 1.0)

            # partition p holds feat rows p*NPT .. p*NPT+NPT-1 (contiguous 8KB)
            fv = feat.ap().rearrange("(p n) d -> p n d", p=128)
            csv = cslice.ap().rearrange("(p n) d -> p n d", p=128)

            for _ in range(repeat):
                # indices first so gathers can start early
                lab = pool.tile([128, NPT], i32, tag="lab")
                nc.sync.dma_start(out=lab[:], in_=labt.ap())

                o_fsq_t = pool.tile([128, NCHUNK], f32, tag="o_fsq_t")
                o_intra_t = pool.tile([128, NCHUNK], f32, tag="o_intra_t")
                o_csq_t = pool.tile([128, 1], f32, tag="o_csq_t")
                vec_sb = pool.tile([1, 1024], f32, tag="vec_sb")

                ps_f = psum.tile([1, CW], f32, tag="ps_f")
                ps_c = psum.tile([1, CW], f32, tag="ps_c")

                # centers-slice statistics (independent of feat path)
                cs_t = pool.tile([128, CSPT * D], f32, tag="cs_t")
                nc.sync.dma_start(out=cs_t[:], in_=csv[:, :, :])
                cs_scr = pool.tile([128, CSPT * D], f32, tag="cs_scr")
                nc.scalar.activation(out=cs_scr[:], in_=cs_t[:],
                                     func=mybir.ActivationFunctionType.Square,
                                     accum_out=o_csq_t[:, 0:1])
                nc.tensor.matmul(out=ps_c[:], lhsT=ones[:], rhs=cs_t[:],
                                 start=True, stop=True)

                for k in range(NCHUNK):
                    f_c = spool.tile([128, CW], f32, tag="f_c")
                    nc.sync.dma_start(out=f_c[:],
                                      in_=fv[:, k * CPC:(k + 1) * CPC, :])
                    cg_c = spool.tile([128, CW], f32, tag="cg_c")
                    if gather_mode == "indirect":
                        for j in range(CPC):
                            nc.gpsimd.indirect_dma_start(
                                out=cg_c[:, j * D:(j + 1) * D],
                                out_offset=None,
                                in_=centers.ap(),
                                in_offset=bass.IndirectOffsetOnAxis(
                                    ap=lab[:, k * CPC + j:k * CPC + j + 1],
                                    axis=0),
                            )
                    else:  # "fake": plain DMA of same volume (timing expt)
                        cv = centers.ap().rearrange(
                            "(q p n) d -> q p n d", p=128, n=CPC)
                        nc.sync.dma_start(out=cg_c[:], in_=cv[k])
                    # sum of f^2 on ACT
                    f_scr = spool.tile([128, CW], f32, tag="f_scr")
                    nc.scalar.activation(
                        out=f_scr[:], in_=f_c[:],
                        func=mybir.ActivationFunctionType.Square,
                        accum_out=o_fsq_t[:, k:k + 1])
                    # column sums of f on PE (accumulated over chunks)
                    nc.tensor.matmul(out=ps_f[:], lhsT=ones[:], rhs=f_c[:],
                                     start=(k == 0), stop=(k == NCHUNK - 1))
                    # intra partial on DVE: d = f - cg; accum += d*d
                    d_c = spool.tile([128, CW], f32, tag="d_c")
                    nc.vector.tensor_sub(d_c[:], f_c[:], cg_c[:])
                    d_scr = spool.tile([128, CW], f32, tag="d_scr")
                    nc.vector.scalar_tensor_tensor(
                        out=d_scr[:], in0=d_c[:], scalar=1.0, in1=d_c[:],
                        op0=mybir.AluOpType.mult, op1=mybir.AluOpType.mult,
                        accum_out=o_intra_t[:, k:k + 1])

                nc.vector.tensor_copy(vec_sb[:, 0:CW], ps_f[:])
                nc.scalar.copy(vec_sb[:, CW:2 * CW], ps_c[:])

                nc.sync.dma_start(out=o_fsq.ap(), in_=o_fsq_t[:])
                nc.sync.dma_start(out=o_intra.ap(), in_=o_intra_t[:])
                nc.sync.dma_start(out=o_csq.ap(), in_=o_csq_t[:])
                nc.sync.dma_start(out=o_vec.ap(), in_=vec_sb[:])

    nc.compile()
    return nc


def _get_nc(repeat=1, gather_mode="indirect"):
    key = ("nc", repeat, gather_mode)
    if key not in _cached:
        _cached[key] = _build_nc(repeat, gather_mode)
    return _cached[key]


def _make_in_maps(feat, label, centers):
    feat = np.ascontiguousarray(np.asarray(feat, dtype=np.float32))
    centers = np.ascontiguousarray(np.asarray(centers, dtype=np.float32))
    lab = np.asarray(label).astype(np.int32)
    in_maps = []
    for k in range(NCORES):
        fs = feat[k * BS:(k + 1) * BS]
        ls = lab[k * BS:(k + 1) * BS].reshape(128, NPT)
        cs = centers[k * CS:(k + 1) * CS]
        in_maps.append({
            "feat": np.ascontiguousarray(fs),
            "labt": np.ascontiguousarray(ls),
            "centers": centers,
            "cslice": np.ascontiguousarray(cs),
        })
    return in_maps


def _combine(results):
    sum_fsq = 0.0
    intra = 0.0
    sum_csq = 0.0
    F = np.zeros(D, dtype=np.float64)
    Cv = np.zeros(D, dtype=np.float64)
    for r in results:
        sum_fsq += r["o_fsq"].astype(np.float64).sum()
        intra += r["o_intra"].astype(np.float64).sum()
        sum_csq += r["o_csq"].astype(np.float64).sum()
        v = r["o_vec"][0].astype(np.float64)
        F += v[:512].reshape(4, 128).sum(axis=0)
        Cv += v[512:].reshape(4, 128).sum(axis=0)
    total = C * sum_fsq + B * sum_csq - 2.0 * float(F @ Cv)
    inter = total - intra
    loss = (LAMBDA_C / 2.0 / B) * intra / (inter + 1e-6) / 0.1
    return np.float32(loss)


def kernel(feat, label, centers):
    from concourse.bass_utils import run_bass_kernel_spmd

    nc = _get_nc()
    in_maps = _make_in_maps(feat, label, centers)
    res = run_bass_kernel_spmd(nc, in_maps, list(range(NCORES)))
    return _combine(res.results)
